# revision 4
# baseline (speedup 1.0000x reference)
"""VQ codebook assignment + nearest upsample on 8 NeuronCores.

Problem (per domain): given features f [B=4, C=256, H=64, W=128] and
centroids c [K=19, C=256], compute argmin_k ||f[b,:,h,w] - c_k||^2 and
nearest-upsample the [64,128] index map to [512,1024] (8x per axis).
Two domains (cross-assigned centroids) x 4 batches = 8 cores, one
batch-image per core, no cross-core communication.

v3 (DMA descriptor-size + store-path rework of the fp16/int32 v2):

  * Features/centroids rounded to fp16 on the host: 1 cycle/row on the
    PE and 4.2 MB/core of input DMA. Measured flip rate vs the fp32
    reference: 0.04% of pixels -> rel_err 1.5e-2, under the 2e-2 gate.
  * Centroids pre-scaled by 256 (exact in fp16); fp32 PSUM scores are
    256*(f.c_k). A bit-exact ScalarE Copy converts them to int32; the
    bias-iota pack B = -32*score + (-32*bq_k + k), ONE DVE min-reduce
    and B & 31 recovers the argmin k with first-match tie semantics.
  * Input DMA: the per-HWDGE-queue bottleneck is DESCRIPTOR DISPATCH
    (~50-60 packets/us/queue), so bandwidth scales with packet size:
    4KB-per-partition pieces sustain ~400 GB/s aggregate over the two
    queues, 1KB pieces only ~100 GB/s. v2 split the last superblock
    into 3KB+1KB pieces, and the 1KB piece (128 x 1KB descriptors)
    trickled for ~4 us at the worst possible time, right before the
    drain-critical sb3 compute. v3 loads 4 equal pieces per channel
    half (4.0-4.2KB/partition each, alternating queues): input is done
    in ~11 us at line rate, ~4.4 us earlier than v2.
  * Stores have the same descriptor economics (512 x 1KB rows for the
    upsampled int8 mask): v3 splits them across THREE dispatchers --
    the h-half-0 store goes on the gpsimd SWDGE ring (Q7 generates
    descriptors in parallel with the HW queues, mid-kernel), and the
    final h-half-1 store is split sync/scalar/gpsimd so all three
    rings dispatch concurrently at the drain.
  * Upsample tail per h-half: DVE 32x32 transpose, then 4 broadcast
    copies (rep[h, 8*(32i+q)+x] = tmp16[32i+p, 32hh+q], int8 out) read
    the block-transposed tile directly. h-half 0's copies run on
    ScalarE (slack mid-kernel, keeps DVE free); h-half 1's split 2 on
    ScalarE + 2 on DVE so the drain-critical replicate halves.
  * The last superblock's compute chain is column-split in two so the
    drain pipelines (copy/transpose/pack/min alternate ScalarE/DVE).

Remaining wall-clock anatomy (per core): ~2.4 us main->first-byte
(handshake + HWDGE descriptor gen + first-byte latency), ~11 us input
DMA at ~400 GB/s, ~4 us drain (sb3 compute + tail + stores), ~8.5 us
fixed runtime teardown (DMA-sem propagation + a 51-per-engine
semaphore-file clear emitted by the PJRT wrapper, outside the kernel).
"""

import numpy as np

import concourse.bass as bass
import concourse.mybir as mybir
import concourse.tile as tile
from concourse import bacc
from concourse.bass import ds
from concourse.bass_utils import run_bass_kernel_spmd

F32 = mybir.dt.float32
F16 = mybir.dt.float16
I32 = mybir.dt.int32
I16 = mybir.dt.int16
I8 = mybir.dt.int8

B = 4
C = 256
H, W = 64, 128
K = 19
KP = 32               # K padded to a 32x32 transpose block
HL, WL = 512, 1024
NPIX = H * W          # 8192
SB = 4                # superblocks (2048 px each)
SBPIX = NPIX // SB
CH = 512              # matmul moving chunk (pixels)
NCH = SBPIX // CH     # chunks per superblock: 4
NJ = CH // KP         # 32-col blocks per score tile: 16
UP = HL // H          # 8x upsample
SC = 256.0            # centroid pre-scale -> int16 score units
FWC = KP + NPIX       # fw columns: [w | pixels]

_NC_CACHE = None


def _build_nc():
    nc = bacc.Bacc("TRN2", target_bir_lowering=False, debug=False)

    fw_in = nc.dram_tensor("fw", [C, FWC], F16, kind="ExternalInput")
    bi_in = nc.dram_tensor("biasiota", [128, KP], I32, kind="ExternalInput")
    mask_out = nc.dram_tensor("mask", [HL, WL], I8, kind="ExternalOutput")

    fwv = fw_in.ap().rearrange("(a p) n -> a p n", a=2)       # [2, 128, FWC]
    outv = mask_out.ap().rearrange("(h y) x -> h y x", y=UP)  # [64, 8, 1024]

    with tile.TileContext(nc) as tc:
        with (
            tc.tile_pool(name="persist", bufs=1) as pp,
            tc.tile_pool(name="work", bufs=3) as wp,
            tc.tile_pool(name="psA", bufs=4, space="PSUM") as psA,
        ):
            fw0 = pp.tile([128, FWC], F16, tag="fw0")
            fw1 = pp.tile([128, FWC], F16, tag="fw1")
            bi32 = pp.tile([128, KP], I32, tag="bi32")
            idxv = pp.tile([128, H], I32, tag="idxv")       # [w, h]
            tmp16 = pp.tile([128, H], I32, tag="tmp16")     # block-transposed
            rep = pp.tile([H, WL], I8, tag="rep")           # x-replicated

            # --- input loads. Per-HWDGE-queue bandwidth is descriptor-
            # dispatch-limited, so pieces are big (4.0-4.2KB per partition
            # -> ~200 GB/s per queue). Pieces arrive in superblock order on
            # alternating queues. sb3 is split in two 2KB halves so the
            # first half's completion semaphore (which lags the data by the
            # ~1us HBM-write-receipt latency) fires while the second half
            # is still in flight. ---
            nc.gpsimd.dma_start(bi32, bi_in[:, :])
            pieces = [
                ds(0, KP + SBPIX),                           # w + sb0
                ds(KP + SBPIX, SBPIX),                       # sb1
                ds(KP + 2 * SBPIX, SBPIX),                   # sb2
                ds(KP + 3 * SBPIX, SBPIX // 2),              # sb3 front
                ds(KP + 3 * SBPIX + SBPIX // 2, SBPIX // 2),  # sb3 back
            ]
            for pi, sl in enumerate(pieces):
                for half in range(2):
                    dst = fw0 if half == 0 else fw1
                    eng = nc.sync if (pi + half) % 2 == 0 else nc.scalar
                    eng.dma_start(dst[:, sl], fwv[half, :, sl])

            # --- per-superblock: 8 matmuls -> int16 scores -> 32x32 block
            # transpose -> packed argmin over k. tile_wait_until hints give
            # the list scheduler a realistic availability timeline (its own
            # DMA model is optimistic, which otherwise sorts the h-half-0
            # tail AFTER sb3's drain-critical chain on Scalar/DVE). ---
            for sb in range(SB):
                psa = psA.tile([64, CH], F32, tag="psa")
                psb = psA.tile([64, CH], F32, tag="psb")
                pst = [psa, psb]
                for cch in range(NCH):
                    colsl = ds(KP + sb * SBPIX + cch * CH, CH)
                    ps = pst[cch // 2]
                    psl = ds(32 * (cch % 2), 32)
                    nc.tensor.matmul(
                        ps[psl, :], fw0[:, 0:KP], fw0[:, colsl],
                        start=True, stop=False,
                    )
                    nc.tensor.matmul(
                        ps[psl, :], fw1[:, 0:KP], fw1[:, colsl],
                        start=False, stop=True,
                    )
                # bit-exact ScalarE Copy: fp32 PSUM -> int32 (RNE)
                St = wp.tile([128, CH], I32, tag="St")
                # DVE 32x32 block transpose -> partition=w, col-block=h,
                # then B = -32*score + (-32*bq_k + k); min over k; k = B&31.
                # The last superblock is column-split in two so its chain
                # pipelines during the drain.
                T = wp.tile([128, CH], I32, tag="T")
                Bt = wp.tile([128, CH], I32, tag="Bt")
                Bm = wp.tile([128, NJ], I32, tag="Bm")
                nsp = 2 if sb == SB - 1 else 1
                cw = CH // nsp
                for cs in range(nsp):
                    t_sb = 0.011 + 0.0028 * sb + 0.0012 * cs
                    csl = ds(cs * cw, cw)
                    with tc.tile_wait_until(t_sb):
                        nc.scalar.copy(St[ds(0, 64), csl], pst[0][:, csl])
                        nc.scalar.copy(St[ds(64, 64), csl], pst[1][:, csl])
                        nc.vector.transpose(T[:, csl], St[:, csl])
                        nc.vector.scalar_tensor_tensor(
                            Bt[:, csl].rearrange("p (j k) -> p j k", k=KP),
                            T[:, csl].rearrange("p (j k) -> p j k", k=KP),
                            -32, bi32.rearrange("p (o k) -> p o k", o=1)
                            .to_broadcast([128, cw // KP, KP]),
                            op0=mybir.AluOpType.mult,
                            op1=mybir.AluOpType.add,
                        )
                        bsl = ds(cs * (cw // KP), cw // KP)
                        nc.vector.tensor_reduce(
                            Bm[:, bsl],
                            Bt[:, csl].rearrange("p (j k) -> p j k", k=KP),
                            axis=mybir.AxisListType.X,
                            op=mybir.AluOpType.min,
                        )
                        nc.vector.tensor_scalar(
                            idxv[:, ds(sb * NJ + cs * (cw // KP), cw // KP)],
                            Bm[:, bsl], 31, None,
                            op0=mybir.AluOpType.bitwise_and,
                        )

                # --- tail, overlapped: after each half of the superblocks,
                # emit that h-half (transpose, x8-replicate, store) ---
                if sb % (SB // 2) != SB // 2 - 1:
                    continue
                hh = sb // (SB // 2)           # 0 or 1
                t_hh = 0.0161 if hh == 0 else 0.0212
                hsl = ds(hh * H // 2, H // 2)  # 32 h columns
                psl = ds(hh * 32, 32)          # matching partition rows
                with tc.tile_wait_until(t_hh):
                    nc.vector.transpose(tmp16[:, hsl], idxv[:, hsl])
                    # 8x replicate along x, int8 out, straight out of the
                    # block-transposed tile (rep[p, 32i+q, x] =
                    # tmp16[32i+p, 32hh+q]). hh0's four copies run on
                    # ScalarE (slack mid-kernel); hh1's split 2 ScalarE +
                    # 2 DVE so the drain-critical replicate halves.
                    repv = rep[psl].rearrange("p (w x) -> p w x", w=W)
                    for i in range(W // 32):
                        tsrc = tmp16[ds(32 * i, 32), hsl].rearrange(
                            "p (q o) -> p q o", o=1
                        ).to_broadcast([32, 32, UP])
                        if hh == 0 or i < 2:
                            nc.scalar.copy(repv[:, ds(32 * i, 32)], tsrc)
                        else:
                            nc.vector.tensor_copy(
                                repv[:, ds(32 * i, 32)], tsrc
                            )
                    # stores: stride-0 source loop re-reads each 1KB SBUF
                    # row 8x for the y-replication. hh0 goes whole on the
                    # gpsimd SWDGE ring (keeps the HW queues clear for
                    # input); the final hh1 store splits sync/scalar/gpsimd
                    # so all three descriptor dispatchers run concurrently
                    # at the drain.
                    if hh == 0:
                        splits = ((nc.gpsimd, 0, 32),)
                    else:
                        splits = (
                            (nc.sync, 0, 12), (nc.scalar, 12, 12),
                            (nc.gpsimd, 24, 8),
                        )
                    for eng, p0, np_ in splits:
                        pssl = ds(hh * 32 + p0, np_)
                        srcap = rep[pssl].rearrange(
                            "p (o x) -> p o x", o=1
                        ).to_broadcast([np_, UP, WL])
                        eng.dma_start(outv[pssl], srcap)

    nc.compile()
    return nc


def _prep_domain(feature, centroid):
    """Per-core inputs for one domain: 4 batches against one centroid set."""
    c = np.asarray(centroid, dtype=np.float64)                  # [K, C]
    w16 = c.T.astype(np.float16)                                # [C, K]
    wsc = (w16.astype(np.float32) * SC).astype(np.float16)      # exact x2^8
    wpad = np.zeros((C, KP), dtype=np.float16)
    wpad[:, :K] = wsc
    c2 = np.sum(c * c, axis=1)                                  # [K]
    bq = np.rint(SC * (c2.mean() - c2) / 2.0).astype(np.int64)
    biasiota = np.full(KP, 2**30, dtype=np.int64)
    biasiota[:K] = -32 * bq + np.arange(K)
    biasiota = np.ascontiguousarray(
        np.tile(biasiota[None, :], (128, 1)), dtype=np.int32
    )
    maps = []
    for b in range(B):
        f16 = np.asarray(feature[b], dtype=np.float32).astype(np.float16)
        # pixel permutation: image (h, w) -> chunk order (sb, cch, h%16, w%32)
        fp = (
            f16.reshape(C, SB, 16, W // 32, 32)
            .transpose(0, 1, 3, 2, 4)
            .reshape(C, NPIX)
        )
        fw = np.ascontiguousarray(np.concatenate([wpad, fp], axis=1))
        maps.append({"fw": fw, "biasiota": biasiota})
    return maps


def kernel(
    feature_s2t, feature_target, label_s2t, label_target,
    centroid_s2t, centroid_target,
):
    global _NC_CACHE
    if _NC_CACHE is None:
        _NC_CACHE = _build_nc()
    nc = _NC_CACHE

    # cross assignment: s2t features vs target centroids, and vice versa
    in_maps = _prep_domain(feature_s2t, centroid_target) + _prep_domain(
        feature_target, centroid_s2t
    )
    res = run_bass_kernel_spmd(nc, in_maps, core_ids=list(range(8))).results
    mask_s2t = np.stack([res[i]["mask"] for i in range(B)]).astype(np.int32)
    mask_target = np.stack([res[B + i]["mask"] for i in range(B)]).astype(
        np.int32
    )
    return (mask_s2t, mask_target)


# revision 11
# speedup vs baseline: 1.0063x; 1.0063x over previous
"""VQ codebook assignment + nearest upsample on 8 NeuronCores.

Problem (per domain): given features f [B=4, C=256, H=64, W=128] and
centroids c [K=19, C=256], compute argmin_k ||f[b,:,h,w] - c_k||^2 and
nearest-upsample the [64,128] index map to [512,1024] (8x per axis).
Two domains (cross-assigned centroids) x 4 batches = 8 cores, one
batch-image per core, no cross-core communication.

v3 (DMA descriptor-size + store-path rework of the fp16/int32 v2):

  * Features/centroids rounded to fp16 on the host: 1 cycle/row on the
    PE and 4.2 MB/core of input DMA. Measured flip rate vs the fp32
    reference: 0.04% of pixels -> rel_err 1.5e-2, under the 2e-2 gate.
  * Centroids pre-scaled by 256 (exact in fp16); fp32 PSUM scores are
    256*(f.c_k). A bit-exact ScalarE Copy converts them to int32; the
    bias-iota pack B = -32*score + (-32*bq_k + k), ONE DVE min-reduce
    and B & 31 recovers the argmin k with first-match tie semantics.
  * Input DMA: the per-HWDGE-queue bottleneck is DESCRIPTOR DISPATCH
    (~50-60 packets/us/queue), so bandwidth scales with packet size:
    4KB-per-partition pieces sustain ~400 GB/s aggregate over the two
    queues, 1KB pieces only ~100 GB/s. v2 split the last superblock
    into 3KB+1KB pieces, and the 1KB piece (128 x 1KB descriptors)
    trickled for ~4 us at the worst possible time, right before the
    drain-critical sb3 compute. v3 loads 4 equal pieces per channel
    half (4.0-4.2KB/partition each, alternating queues): input is done
    in ~11 us at line rate, ~4.4 us earlier than v2.
  * Stores have the same descriptor economics (512 x 1KB rows for the
    upsampled int8 mask): v3 splits them across THREE dispatchers --
    the h-half-0 store goes on the gpsimd SWDGE ring (Q7 generates
    descriptors in parallel with the HW queues, mid-kernel), and the
    final h-half-1 store is split sync/scalar/gpsimd so all three
    rings dispatch concurrently at the drain.
  * Upsample tail per h-half: DVE 32x32 transpose, then 4 broadcast
    copies (rep[h, 8*(32i+q)+x] = tmp16[32i+p, 32hh+q], int8 out) read
    the block-transposed tile directly. h-half 0's copies run on
    ScalarE (slack mid-kernel, keeps DVE free); h-half 1's split 2 on
    ScalarE + 2 on DVE so the drain-critical replicate halves.
  * The last superblock's compute chain is column-split in two so the
    drain pipelines (copy/transpose/pack/min alternate ScalarE/DVE).

Remaining wall-clock anatomy (per core): ~2.4 us main->first-byte
(handshake + HWDGE descriptor gen + first-byte latency), ~11 us input
DMA at ~400 GB/s, ~4 us drain (sb3 compute + tail + stores), ~8.5 us
fixed runtime teardown (DMA-sem propagation + a 51-per-engine
semaphore-file clear emitted by the PJRT wrapper, outside the kernel).
"""

import numpy as np

import concourse.bass as bass
import concourse.mybir as mybir
import concourse.tile as tile
from concourse import bacc
from concourse.bass import ds
from concourse.bass_utils import run_bass_kernel_spmd

F32 = mybir.dt.float32
F16 = mybir.dt.float16
I32 = mybir.dt.int32
I16 = mybir.dt.int16
I8 = mybir.dt.int8

B = 4
C = 256
H, W = 64, 128
K = 19
KP = 32               # K padded to a 32x32 transpose block
HL, WL = 512, 1024
NPIX = H * W          # 8192
SB = 4                # superblocks (2048 px each)
SBPIX = NPIX // SB
CH = 512              # matmul moving chunk (pixels)
NCH = SBPIX // CH     # chunks per superblock: 4
NJ = CH // KP         # 32-col blocks per score tile: 16
UP = HL // H          # 8x upsample
SC = 256.0            # centroid pre-scale -> int16 score units
FWC = KP + NPIX       # fw columns: [w | pixels]

_NC_CACHE = None


def _build_nc():
    nc = bacc.Bacc("TRN2", target_bir_lowering=False, debug=False)

    fw_in = nc.dram_tensor("fw", [C, FWC], F16, kind="ExternalInput")
    bi_in = nc.dram_tensor("biasiota", [128, 1], F32, kind="ExternalInput")
    # mask as int16 [512, 512]: each int16 is a replicated byte PAIR of the
    # int8 mask, so one store descriptor covers 2 output rows (2KB) -- the
    # per-queue store bottleneck is descriptor dispatch, not bytes. The
    # host views the buffer back as int8 [512, 1024].
    mask_out = nc.dram_tensor("mask", [HL, WL // 2], I16, kind="ExternalOutput")

    fwv = fw_in.ap().rearrange("(a p) n -> a p n", a=2)       # [2, 128, FWC]
    # dst rows r = 8h + 2v + y: partition h, 4 descriptors of 2KB each
    outv = mask_out.ap().rearrange("(h v y) x -> h v (y x)", v=4, y=2)

    with tile.TileContext(nc) as tc:
        with (
            tc.tile_pool(name="persist", bufs=1) as pp,
            tc.tile_pool(name="work", bufs=3) as wp,
            tc.tile_pool(name="psA", bufs=4, space="PSUM") as psA,
        ):
            fw0 = pp.tile([128, FWC], F16, tag="fw0")
            fw1 = pp.tile([128, FWC], F16, tag="fw1")
            bi32 = pp.tile([128, 1], F32, tag="bi32")
            idxv = pp.tile([128, H], I32, tag="idxv")       # [w, h]
            tmp16 = pp.tile([128, H], I32, tag="tmp16")     # block-transposed
            rep = pp.tile([H, WL], I16, tag="rep")  # 2 copies of xrep row

            # --- input loads. Per-HWDGE-queue bandwidth is descriptor-
            # dispatch-limited, so pieces are big (4.0-4.2KB per partition
            # -> ~200 GB/s per queue). Pieces arrive in superblock order on
            # alternating queues. sb3 is split in two 2KB halves so the
            # first half's completion semaphore (which lags the data by the
            # ~1us HBM-write-receipt latency) fires while the second half
            # is still in flight. ---
            nc.gpsimd.dma_start(bi32, bi_in[:, :])
            pieces = [
                ds(0, KP + SBPIX),                           # w + sb0
                ds(KP + SBPIX, SBPIX),                       # sb1
                ds(KP + 2 * SBPIX, SBPIX),                   # sb2
                ds(KP + 3 * SBPIX, SBPIX // 2),              # sb3 front
                ds(KP + 3 * SBPIX + SBPIX // 2, SBPIX // 2),  # sb3 back
            ]
            for pi, sl in enumerate(pieces):
                for half in range(2):
                    dst = fw0 if half == 0 else fw1
                    eng = nc.sync if (pi + half) % 2 == 0 else nc.scalar
                    eng.dma_start(dst[:, sl], fwv[half, :, sl])

            # --- per-superblock: 8 matmuls -> int16 scores -> 32x32 block
            # transpose -> packed argmin over k. tile_wait_until hints give
            # the list scheduler a realistic availability timeline (its own
            # DMA model is optimistic, which otherwise sorts the h-half-0
            # tail AFTER sb3's drain-critical chain on Scalar/DVE). ---
            for sb in range(SB):
                psa = psA.tile([64, CH], F32, tag="psa")
                psb = psA.tile([64, CH], F32, tag="psb")
                pst = [psa, psb]
                for cch in range(NCH):
                    colsl = ds(KP + sb * SBPIX + cch * CH, CH)
                    ps = pst[cch // 2]
                    psl = ds(32 * (cch % 2), 32)
                    nc.tensor.matmul(
                        ps[psl, :], fw0[:, 0:KP], fw0[:, colsl],
                        start=True, stop=False,
                    )
                    nc.tensor.matmul(
                        ps[psl, :], fw1[:, 0:KP], fw1[:, colsl],
                        start=False, stop=True,
                    )
                # bit-exact ScalarE Copy: fp32 PSUM -> int32 (RNE), then the
                # ScalarE Identity pack B = -32*St + (-32*bq_k + k) with the
                # bias-iota as a per-partition vector (partition = k here,
                # pre-transpose). Identity's LUT path is measured bit-exact
                # for integer-valued fp32 in +-2^22, and every term is an
                # exact fp32 integer, so B is exact and k = B & 31 after the
                # min survives. Keeping the pack on ScalarE slims the DVE
                # chain to transpose+min+and.
                St = wp.tile([128, CH], I32, tag="St")
                Bp = wp.tile([128, CH], I32, tag="Bp")
                T = wp.tile([128, CH], I32, tag="T")
                Bm = wp.tile([128, NJ], I32, tag="Bm")
                nsp = 2 if sb == SB - 1 else 1
                cw = CH // nsp
                for cs in range(nsp):
                    t_sb = (0.0118, 0.0144, 0.0171, 0.0193)[sb] + 0.0012 * cs
                    csl = ds(cs * cw, cw)
                    with tc.tile_wait_until(t_sb):
                        nc.scalar.copy(St[ds(0, 64), csl], pst[0][:, csl])
                        nc.scalar.copy(St[ds(64, 64), csl], pst[1][:, csl])
                        nc.scalar.activation(
                            Bp[:, csl], St[:, csl],
                            mybir.ActivationFunctionType.Identity,
                            bias=bi32[:, 0:1], scale=-32.0,
                        )
                        nc.vector.transpose(T[:, csl], Bp[:, csl])
                        bsl = ds(cs * (cw // KP), cw // KP)
                        nc.vector.tensor_reduce(
                            Bm[:, bsl],
                            T[:, csl].rearrange("p (j k) -> p j k", k=KP),
                            axis=mybir.AxisListType.X,
                            op=mybir.AluOpType.min,
                        )
                        nc.vector.tensor_scalar(
                            idxv[:, ds(sb * NJ + cs * (cw // KP), cw // KP)],
                            Bm[:, bsl], 31, None,
                            op0=mybir.AluOpType.bitwise_and,
                        )

                # --- tail, overlapped: after each half of the superblocks,
                # emit that h-half (transpose, x8-replicate, store) ---
                if sb % (SB // 2) != SB // 2 - 1:
                    continue
                hh = sb // (SB // 2)           # 0 or 1
                t_hh = 0.0188 if hh == 0 else 0.0212
                hsl = ds(hh * H // 2, H // 2)  # 32 h columns
                psl = ds(hh * 32, 32)          # matching partition rows
                with tc.tile_wait_until(t_hh):
                    nc.vector.transpose(tmp16[:, hsl], idxv[:, hsl])
                    # x-replicate straight out of the block-transposed tile
                    # (value at [p, w=32i+q] is tmp16[32i+p, 32hh+q]), as
                    # int16 byte-pairs: out = idx * 257 packs (b, b) per
                    # int16, so 4 int16 steps cover the 8 int8 output bytes
                    # and the freed column budget emits TWO copies of the
                    # 1KB row -> 2KB store descriptors at the same op cost.
                    # hh0's four copies run on ScalarE (slack mid-kernel);
                    # hh1's split 2 ScalarE + 2 DVE.
                    repv = rep[psl].rearrange(
                        "p (u w x) -> p u w x", u=2, x=UP // 2
                    )
                    for i in range(W // 32):
                        tsrc = tmp16[ds(32 * i, 32), hsl].rearrange(
                            "p (a q o) -> p a q o", a=1, o=1
                        ).to_broadcast([32, 2, 32, UP // 2])
                        dst = repv[:, :, ds(32 * i, 32), :]
                        if hh == 0 or i < 2:
                            nc.scalar.activation(
                                dst, tsrc,
                                mybir.ActivationFunctionType.Identity,
                                scale=257.0,
                            )
                        else:
                            nc.vector.tensor_scalar(
                                dst, tsrc, 257, None,
                                op0=mybir.AluOpType.mult,
                            )
                    # stores: stride-0 source loop re-reads each 2KB SBUF
                    # row 4x for the y-replication (descriptor = 2 output
                    # rows). hh0 goes whole on the gpsimd SWDGE ring (keeps
                    # the HW queues clear for input); the final hh1 store
                    # splits sync/scalar/gpsimd so all three descriptor
                    # dispatchers run concurrently at the drain.
                    if hh == 0:
                        splits = ((nc.gpsimd, 0, 32),)
                    else:
                        splits = (
                            (nc.sync, 0, 12), (nc.scalar, 12, 12),
                            (nc.gpsimd, 24, 8),
                        )
                    for eng, p0, np_ in splits:
                        pssl = ds(hh * 32 + p0, np_)
                        srcap = rep[pssl].rearrange(
                            "p (o c) -> p o c", o=1
                        ).to_broadcast([np_, 4, WL])
                        eng.dma_start(outv[pssl], srcap)

    nc.compile()
    return nc


def _prep_domain(feature, centroid):
    """Per-core inputs for one domain: 4 batches against one centroid set."""
    c = np.asarray(centroid, dtype=np.float64)                  # [K, C]
    w16 = c.T.astype(np.float16)                                # [C, K]
    wsc = (w16.astype(np.float32) * SC).astype(np.float16)      # exact x2^8
    wpad = np.zeros((C, KP), dtype=np.float16)
    wpad[:, :K] = wsc
    c2 = np.sum(c * c, axis=1)                                  # [K]
    bq = np.rint(SC * (c2.mean() - c2) / 2.0).astype(np.int64)
    # per-partition pack bias (partition = 32*cch + k pre-transpose):
    # -32*bq_k + k for real k, +2^22 for the zero-weight pad rows (their
    # scores are exactly 0, so B_pad = 2^22 > any real B, never the min).
    # All values are integers < 2^22, exact in fp32.
    biasiota = np.full(KP, 2**22, dtype=np.int64)
    biasiota[:K] = -32 * bq + np.arange(K)
    biasiota = np.ascontiguousarray(
        np.tile(biasiota, 4)[:, None], dtype=np.float32
    )
    maps = []
    for b in range(B):
        f16 = np.asarray(feature[b], dtype=np.float32).astype(np.float16)
        # pixel permutation: image (h, w) -> chunk order (sb, cch, h%16, w%32)
        fp = (
            f16.reshape(C, SB, 16, W // 32, 32)
            .transpose(0, 1, 3, 2, 4)
            .reshape(C, NPIX)
        )
        fw = np.ascontiguousarray(np.concatenate([wpad, fp], axis=1))
        maps.append({"fw": fw, "biasiota": biasiota})
    return maps


def kernel(
    feature_s2t, feature_target, label_s2t, label_target,
    centroid_s2t, centroid_target,
):
    global _NC_CACHE
    if _NC_CACHE is None:
        _NC_CACHE = _build_nc()
    nc = _NC_CACHE

    # cross assignment: s2t features vs target centroids, and vice versa
    in_maps = _prep_domain(feature_s2t, centroid_target) + _prep_domain(
        feature_target, centroid_s2t
    )
    res = run_bass_kernel_spmd(nc, in_maps, core_ids=list(range(8))).results
    # device writes int16 byte-pairs [512, 512]; view back as int8 [512,1024]
    masks = [
        np.ascontiguousarray(res[i]["mask"]).view(np.int8).reshape(HL, WL)
        for i in range(2 * B)
    ]
    mask_s2t = np.stack(masks[:B]).astype(np.int32)
    mask_target = np.stack(masks[B:]).astype(np.int32)
    return (mask_s2t, mask_target)
